# revision 74
# baseline (speedup 1.0000x reference)
"""Enformer-style relative-position attention (nn_Attention_27925877358942) for
8 Trainium2 NeuronCores.

Contract: kernel(**inputs) takes the FULL unsharded inputs (keys as in
setup_inputs()) and returns the full [1, 4096, 1536] float32 output.

Sharding: one head per core (8 heads / 8 cores). Host precomputes the
deterministic positional-feature table and x^T in fp16, slices per-head
weights, runs the SPMD Bass kernel via run_bass_kernel_spmd, and combines the
per-head partial outputs (+ b_out).

Two exact host-side refactorings of the math carry most of the win:
  - content-bias fold: exp(rel_content_bias . k_j) is a per-key factor of
    the softmax numerator; the host computes it exactly as
    exp(x @ (W_k @ bc) - C) and the device folds it into the v rows and the
    denominator ("ones") column of vext. The device-side content logits are
    then just (q.k)*scale (small values), and the output partials/row sums
    ship unnormalized (bf16/f32); the host divides and sums over heads.
  - softmax normalization on host: PV accumulates [o | rowsum] in one PSUM
    pass; both ship out, so the device epilogue has no reciprocal pass.

Device pipeline per core (head h), N=4096, d=64 (all tiles run the fused
"A-mode" path; a split-exp B-mode and fp8e4-DoubleRow matmul variants exist
behind flags but lose accuracy or win no time at the current balance):
  - q^T,k^T (fp16, [64,N]) and vext = [e^bck*v | e^bck] projections on PE
  - r^T = (pos @ Wrelk_h)^T rel table (fp16)
  - per q-tile I: window logits em[di,c] = (q_i*scale+bp).r[t0+c] on PE;
    copied PSUM->SBUF as fp8e3 LOGITS by DVE (fp8e3 is safe in logit space:
    |logits| <= ~8 << 15.5 max, underflow near 0 means weight ~ 1)
  - relative_shift via fp8e3 DRAM roundtrip (half the DMA bytes of bf16):
    sheared strided read shr[di,j] = em[di, 127-di+j] (partition stride =
    rowpitch-1 elements)
  - content logits k.q accumulate in PSUM per key-tile block and the
    SHIFTED rel logits are ADDED into the same PSUM region by a plain
    matmul with identity as the moving operand (lhsT=shr block -> +=shr^T);
    ONE ACT Exp pass produces p^T directly (single exp per logit, ACT is
    the scarce engine for exps; em-copies go to DVE, out-copies to ACT —
    homogeneous per-queue instruction streams schedule best)
  - O = pT.T @ vext accumulated in PSUM; transpose o, W_out matmul, ship
    bf16 partials + f32 row sums; host normalizes and sums over heads.

Schedule notes: the Tile list scheduler canonicalizes instruction order, so
throughput is controlled via pool buffer counts (PSUM: em 3x[128,512] +
content 3x[128,512] + epilogue 2 banks) rather than emission order. All
DMAs issue from the SP queue (splitting queues measured worse).

This walrus build accepts at most ONE sync wait per instruction, so after
Tile scheduling every multi-wait instruction is split by inserting
wait-carrying NoOps just before it on the same engine (split_multi_waits),
and the Tile tail drain is built with the same constraint.
"""


_DRAIN_PATCHED = [False]


def _patch_tile_drain():
    if _DRAIN_PATCHED[0]:
        return
    _DRAIN_PATCHED[0] = True
    import concourse.tile as tile_mod
    from concourse.vector_clock import ScopedClock

    MAX_WAITS = 1

    def _drain_and_barrier(self, tick_clock, wait_clock):
        nc = self.nc
        drain_inst = nc.sync.drain()
        wait_clock.add_sem_waits(drain_inst.ins, ScopedClock({None: tick_clock.global_clock}))
        si = drain_inst.ins.sync_info
        waits = list(si.on_wait) if si is not None and si.on_wait else []
        if len(waits) > MAX_WAITS:
            si.on_wait = waits[:MAX_WAITS]
            rest = waits[MAX_WAITS:]
            import concourse.mybir as _mb
            for i in range(0, len(rest), MAX_WAITS):
                extra = nc.sync.drain()
                esi = extra.ins.sync_info
                if esi is None:
                    extra.ins.sync_info = _mb.SyncInfo(on_wait=rest[i:i + MAX_WAITS], on_update=[])
                else:
                    esi.on_wait = rest[i:i + MAX_WAITS]
        nc.all_engine_barrier()
        assert self.sems is not None
        popped = nc._tile_sem_poison_stack.pop()
        assert popped is self._sem_poison
        nc.clear_and_free_semaphores(list(self.sems.allocated().values()))
        nc.all_engine_barrier()

    tile_mod.TileContext._drain_and_barrier = _drain_and_barrier


def split_multi_waits(nc):
    """This walrus build allows at most ONE sync wait per instruction.
    Move extra waits onto InstNoOp carriers inserted just before, on the
    same engine queue (sequencers execute in order, so semantics hold)."""
    import concourse.mybir as mb
    n_split = 0
    for fn in nc.m.functions:
        for bb in fn.blocks:
            insts = list(bb.instructions)
            out = []
            for inst in insts:
                si = inst.sync_info
                waits = list(si.on_wait) if si is not None and si.on_wait else []
                if len(waits) > 1:
                    for w in waits[:-1]:
                        n_split += 1
                        nop = mb.InstNoOp(
                            name=f"waitsplit-{n_split}",
                            engine=inst.engine,
                            sync_info=mb.SyncInfo(on_wait=[w], on_update=[]),
                        )
                        out.append(nop)
                    si.on_wait = [waits[-1]]
                out.append(inst)
            if len(out) != len(insts):
                bb.instructions[:] = out
    return n_split


import math
from contextlib import ExitStack

import numpy as np

import concourse.bass as bass
import concourse.tile as tile
from concourse import mybir
from concourse.bass import ts, ds
from concourse.masks import make_identity

F32 = mybir.dt.float32
BF16 = mybir.dt.bfloat16
FP16 = mybir.dt.float16
FP8 = mybir.dt.float8e4
FP8E3 = mybir.dt.float8e3
AF = mybir.ActivationFunctionType
DR = mybir.MatmulPerfMode.DoubleRow

DIM = 1536
H = 8
D = 64

# ---- tunables (env-overridable for sweeps) ----
import os as _os

def _env(name, default):
    v = _os.environ.get(name)
    return type(default)(v) if v is not None else default

N_A_PAIRS = _env("K_A_PAIRS", 16)  # pairs in A-mode (fused exp)
FP8_DR_CONTENT = bool(_env("K_FP8_CT", 0))
FP8_DR_EM = bool(_env("K_FP8_EM", 0))
EM_SHIFT = 3.75        # B-mode: exp(rel - EM_SHIFT) to fit fp8e4 range
B_CAST_READ = bool(_env("K_BCAST", 0))
A_FP8_SHEAR = bool(_env("K_A_FP8", 1))   # A-mode: shear LOGITS in fp8e3
# GPSIMD cannot read PSUM on real HW: PSUM->SBUF copies go to ACT or DVE.
# Homogeneous queues schedule best: em copies all-DVE, out copies all-ACT.
A_COPY_ACT_MOD = _env("K_ACT_MOD", 99)   # every k-th A em-copy chunk -> ACT
OUT_COPY_ACT_MOD = _env("K_OUT_MOD", 1)  # every k-th out-copy chunk -> ACT
HOST_NORM = bool(_env("K_HOST_NORM", 1))  # softmax row-normalize on host
EM_WRITE_SWDGE = bool(_env("K_EMW_SWDGE", 0))
SHEAR_Q = _os.environ.get("K_SHEAR_Q", "sp")   # sp | act | pool
OUT_Q = _os.environ.get("K_OUT_Q", "sp")       # sp | act | pool
EMW_Q = _os.environ.get("K_EMW_Q", "sp")       # sp | act | pool
CT_BIG = bool(_env("K_CT_BIG", 0))   # content psum [128,1024] shared w/ ppool_m
WSHEAR_A_BUFS = _env("K_WSA", 4)
WSHEAR_B_BUFS = _env("K_WSB", 4)
WORK_BUFS = _env("K_WORK", 2)
EPI_BUFS = _env("K_EPI", 2)
PIPE_DEPTH = _env("K_PIPE", 1)
INTERLEAVE = _env("K_IL", 2)   # attn steps pumped per em step
PROBE = _os.environ.get("K_PROBE", "")  # sim-only sensitivity probes
PREP_OFFLOAD = bool(_env("K_PREP_OFF", 0))  # k/vext/rT copies off ACT


def a_pair_set(n_pairs, n_a):
    if not n_a:
        return set()
    return {min(n_pairs - 1, round(i * n_pairs / n_a)) for i in range(n_a)}


def build(N, split_waits=True):
    Q = N // 128           # query tiles
    NJ = N // 128          # key tiles
    PW = 2 * N             # padded positional width (2n-1 real cols + 1 pad)
    WN = N + 128           # rel window width per q-tile (incl. 1 pad col)
    KD = DIM // 128        # contraction tiles for projections

    a_pairs = a_pair_set(Q // 2, N_A_PAIRS)
    n_a = 2 * len(a_pairs)
    n_b = Q - n_a

    nc = bass.Bass("TRN2", target_bir_lowering=False, debug=False)

    xT_d = nc.dram_tensor("xT", [DIM, N], FP16, kind="ExternalInput")
    posT_d = nc.dram_tensor("posT", [192, PW], FP16, kind="ExternalInput")
    wq_d = nc.dram_tensor("wq", [DIM, D], FP16, kind="ExternalInput")
    wk_d = nc.dram_tensor("wk", [DIM, D], FP16, kind="ExternalInput")
    wv_d = nc.dram_tensor("wv", [DIM, D], FP16, kind="ExternalInput")
    wrk_d = nc.dram_tensor("wrk", [192, D], FP16, kind="ExternalInput")
    wo_d = nc.dram_tensor("wo", [D, DIM], BF16, kind="ExternalInput")
    bc_d = nc.dram_tensor("bc", [D, 1], F32, kind="ExternalInput")
    bp_d = nc.dram_tensor("bp", [D, 1], F32, kind="ExternalInput")
    # content-bias trick: exp(rel_content_bias . k_j) is a per-key factor,
    # host-computed exactly as exp(x @ (W_k @ bc) - C) and folded into the
    # v / ones columns of vext, so the fp8 content matmul only carries the
    # small (q.k) term. Output is then unnormalized at e^bck scale -> bf16.
    ebck_d = nc.dram_tensor("ebck", [128, N // 128], F32, kind="ExternalInput")
    out_d = nc.dram_tensor("out", [N, DIM], BF16, kind="ExternalOutput")
    sums_d = nc.dram_tensor("sums", [128, N // 128], F32, kind="ExternalOutput")
    a_sh_t = FP8E3 if A_FP8_SHEAR else BF16
    em16_d = em8_d = None
    if n_a:
        em16_d = nc.dram_tensor("ema_scratch", [n_a * 128, WN], a_sh_t, kind="Internal")
    if n_b:
        em8_d = nc.dram_tensor("em8_scratch", [n_b * 128, WN], FP8, kind="Internal")

    scale = D ** -0.5

    with tile.TileContext(nc) as tc, ExitStack() as ctx:
        consts = ctx.enter_context(tc.tile_pool(name="consts", bufs=1))
        persist = ctx.enter_context(tc.tile_pool(name="persist", bufs=1))

        # ---- constants ----
        ident = consts.tile([128, 128], BF16, tag="ident")
        make_identity(nc, ident[:])
        if A_FP8_SHEAR:
            ident8 = consts.tile([128, 128], FP8E3, tag="ident8")
            nc.vector.tensor_copy(ident8[:], ident[:])
        else:
            ident8 = ident
        bc_sb = consts.tile([D, 1], F32, tag="bc")
        nc.sync.dma_start(out=bc_sb[:], in_=bc_d.ap())
        bp_sb = consts.tile([D, 1], F32, tag="bp")
        nc.sync.dma_start(out=bp_sb[:], in_=bp_d.ap())
        wo_sb = consts.tile([D, DIM], BF16, tag="wo")
        nc.sync.dma_start(out=wo_sb[:], in_=wo_d.ap())
        negc_sb = consts.tile([128, 1], F32, tag="negc")
        nc.vector.memset(negc_sb[:], -EM_SHIFT)
        ebck_sb = consts.tile([128, NJ], F32, tag="ebck")
        nc.sync.dma_start(out=ebck_sb[:], in_=ebck_d.ap())

        wqk_sb = consts.tile([128, KD, 2 * D], FP16, tag="wqk")
        wv_sb = consts.tile([128, KD, D], FP16, tag="wv")
        nc.sync.dma_start(out=wqk_sb[:, :, 0:D],
                          in_=wq_d.ap().rearrange("(t p) c -> p t c", p=128))
        nc.sync.dma_start(out=wqk_sb[:, :, D:2 * D],
                          in_=wk_d.ap().rearrange("(t p) c -> p t c", p=128))
        nc.sync.dma_start(out=wv_sb[:],
                          in_=wv_d.ap().rearrange("(t p) c -> p t c", p=128))
        wrk_sb = consts.tile([96, 2, D], FP16, tag="wrk")
        for u in range(2):
            nc.sync.dma_start(out=wrk_sb[:, u, :], in_=wrk_d[ts(u, 96), :])

        # ---- persistent activations ----
        # fp8 DoubleRow layout: contraction d = u*32+p on [32 partitions, 2].
        # qp | qc | k packed along the free axis of one [32, 2, 3N] tile.
        any_fp8 = FP8_DR_EM or FP8_DR_CONTENT
        if any_fp8:
            qpk8 = persist.tile([32, 2, 3 * N], FP8, tag="qpk8")
        if FP8_DR_EM:
            rT8 = persist.tile([32, 2, PW], FP8, tag="rT8")
        else:
            qpT = persist.tile([D, N], FP16, tag="qpT")
            rT = persist.tile([D, PW], FP16, tag="rT")
        if not FP8_DR_CONTENT:
            qcT = persist.tile([D, N], FP16, tag="qcT")
            kT = persist.tile([D, N], FP16, tag="kT")
        vext = persist.tile([128, NJ * (D + 1)], BF16, tag="vext")
        sums_sb = persist.tile([128, Q], F32, tag="sums")

        # ---- phases 1-2: rel-k table + projections ----
        with tc.tile_pool(name="stream", bufs=1) as stream, \
             tc.tile_pool(name="prep_psum", bufs=2, space="PSUM") as prep_psum:
            # rel-k table first: independent of x, runs during the xT load
            pall = stream.tile([96, 2, PW], FP16, tag="pall")
            nc.sync.dma_start(out=pall[:, 0, :], in_=posT_d[0:96, :])
            nc.sync.dma_start(out=pall[:, 1, :], in_=posT_d[96:192, :])
            for rc in reversed(range(PW // 512)):
                c0 = rc * 512
                pc = pall[:, :, ds(c0, 512)]
                ps_r = prep_psum.tile([D, 512], F32, tag="ps_qk")
                for u in range(2):
                    nc.tensor.matmul(
                        ps_r[:], wrk_sb[:, u, :], pc[:, u, :],
                        start=(u == 0), stop=(u == 1),
                    )
                if FP8_DR_EM:
                    for u in range(2):
                        eng = nc.vector if (PREP_OFFLOAD and u == 0) else nc.scalar
                        if eng is nc.vector:
                            eng.tensor_copy(rT8[:, u, ds(c0, 512)], ps_r[ts(u, 32), :])
                        else:
                            eng.copy(out=rT8[:, u, ds(c0, 512)], in_=ps_r[ts(u, 32), :])
                else:
                    nc.scalar.copy(out=rT[:, ds(c0, 512)], in_=ps_r[:])

            xT_v = xT_d.ap().rearrange("(t p) n -> p t n", p=128)
            NXS = 4   # independent xT slices so projections start early
            xslices = []
            for sx in range(NXS):
                h0 = sx * (N // NXS)
                xs = stream.tile([128, KD, N // NXS], FP16, tag=f"xall{sx}")
                nc.sync.dma_start(out=xs[:], in_=xT_v[:, :, ds(h0, N // NXS)])
                xslices.append(xs)
            for ic in range(N // 512):
                i0 = ic * 512
                sx = i0 // (N // NXS)
                xc = xslices[sx][:, :, ds(i0 - sx * (N // NXS), 512)]
                ps_qk = prep_psum.tile([128, 512], F32, tag="ps_qk")
                for kd in range(KD):
                    nc.tensor.matmul(
                        ps_qk[:], wqk_sb[:, kd, :], xc[:, kd, :],
                        start=(kd == 0), stop=(kd == KD - 1),
                    )
                if FP8_DR_EM:
                    for u in range(2):
                        nc.scalar.activation(
                            out=qpk8[:, u, ds(i0, 512)],
                            in_=ps_qk[ts(u, 32), :], func=AF.Identity,
                            bias=bp_sb[ts(u, 32), :], scale=scale,
                        )
                else:
                    nc.scalar.activation(
                        out=qpT[:, ds(i0, 512)], in_=ps_qk[0:D, :], func=AF.Identity,
                        bias=bp_sb[:], scale=scale,
                    )
                if FP8_DR_CONTENT:
                    for u in range(2):
                        nc.scalar.activation(
                            out=qpk8[:, u, ds(N + i0, 512)],
                            in_=ps_qk[ts(u, 32), :], func=AF.Identity,
                            scale=scale,
                        )
                        if PREP_OFFLOAD:
                            nc.vector.tensor_copy(
                                qpk8[:, u, ds(2 * N + i0, 512)],
                                ps_qk[D + u * 32:D + (u + 1) * 32, :],
                            )
                        else:
                            nc.scalar.copy(
                                out=qpk8[:, u, ds(2 * N + i0, 512)],
                                in_=ps_qk[D + u * 32:D + (u + 1) * 32, :],
                            )
                else:
                    nc.scalar.activation(
                        out=qcT[:, ds(i0, 512)], in_=ps_qk[0:D, :], func=AF.Identity,
                        scale=scale,
                    )
                    if PREP_OFFLOAD:
                        nc.vector.tensor_copy(kT[:, ds(i0, 512)], ps_qk[D:2 * D, :])
                    else:
                        nc.scalar.copy(out=kT[:, ds(i0, 512)], in_=ps_qk[D:2 * D, :])
                for isb in range(4):
                    J = ic * 4 + isb
                    ps_v = prep_psum.tile([128, D], F32, tag="ps_v")
                    for kd in range(KD):
                        nc.tensor.matmul(
                            ps_v[:], xc[:, kd, ts(isb, 128)], wv_sb[:, kd, :],
                            start=(kd == 0), stop=(kd == KD - 1),
                        )
                    # scale v rows and the softmax-denominator column by the
                    # per-key content-bias factor e^(bc.k_j - C)
                    nc.vector.tensor_scalar_mul(
                        vext[:, ds(J * (D + 1), D)], ps_v[:], ebck_sb[:, J:J + 1])
                    nc.vector.tensor_copy(
                        vext[:, ds(J * (D + 1) + D, 1)], ebck_sb[:, J:J + 1])

        def em_matmul(ps_slice, i0, w0, sw):
            """rel-window logits for q-tile at i0, window cols [w0, w0+sw)."""
            if FP8_DR_EM:
                nc.tensor.matmul(
                    ps_slice, qpk8[:, :, ds(i0, 128)], rT8[:, :, ds(w0, sw)],
                    perf_mode=DR, start=True, stop=True,
                )
            else:
                nc.tensor.matmul(
                    ps_slice, qpT[:, ds(i0, 128)], rT[:, ds(w0, sw)],
                    start=True, stop=True,
                )

        def content_matmul(ps_slice, J, q0, width, start, stop):
            """content logits^T: keys J-tile x query cols [q0, q0+width)."""
            if FP8_DR_CONTENT:
                nc.tensor.matmul(
                    ps_slice, qpk8[:, :, ds(2 * N + J * 128, 128)],
                    qpk8[:, :, ds(N + q0, width)],
                    perf_mode=DR, start=start, stop=stop,
                )
            else:
                nc.tensor.matmul(
                    ps_slice, kT[:, ts(J, 128)], qcT[:, ds(q0, width)],
                    start=start, stop=stop,
                )

        # ---- phase 3: main loop, q-tiles in pairs ----
        work = ctx.enter_context(tc.tile_pool(name="work", bufs=WORK_BUFS))
        ect_pool = ctx.enter_context(tc.tile_pool(name="ect", bufs=3))
        sm = ctx.enter_context(tc.tile_pool(name="sm", bufs=3))
        MW = _env("K_MW", 512)
        ppool_m = ctx.enter_context(
            tc.tile_pool(name="ppool_m", bufs=_env("K_MBUFS", 3), space="PSUM"))
        ppool_ct = ctx.enter_context(
            tc.tile_pool(name="ppool_ct", bufs=_env("K_CTBUFS", 3), space="PSUM"))
        if n_b:
            ppool_st = ctx.enter_context(tc.tile_pool(name="ppool_st", bufs=1, space="PSUM"))
        ppool_epi = ctx.enter_context(tc.tile_pool(name="ppool_epi", bufs=EPI_BUFS, space="PSUM"))
        wshear_a = ctx.enter_context(tc.tile_pool(name="wshear_a", bufs=WSHEAR_A_BUFS)) if n_a else None
        wshear_b = ctx.enter_context(tc.tile_pool(name="wshear_b", bufs=WSHEAR_B_BUFS)) if n_b else None

        b_shr_t = BF16 if B_CAST_READ else FP8

        a_slot = [0]
        b_slot = [0]
        n_copy = [0]

        def em_phase(g, shr_pair):
            i0g = g * 256
            mode_a = g in a_pairs
            for q in range(2):
                I = 2 * g + q
                i0 = I * 128
                t0 = N - 1 - i0 - 127

                if mode_a:
                    em_sb = wshear_a.tile([128, WN], a_sh_t, tag="em16")
                else:
                    em_sb = wshear_b.tile([128, WN], FP8, tag="em8")
                n_full = (WN - 128) // MW
                chunks = [(c * MW, MW) for c in range(n_full)]
                chunks.append((n_full * MW, WN - 1 - n_full * MW))
                for (c0, cw) in chunks:
                    ps = ppool_m.tile([128, MW], F32, tag="ps_m")
                    for s0 in range(0, cw, 512):
                        sw = min(512, cw - s0)
                        if PROBE == "em":
                            sw = 32
                        em_matmul(ps[:, ds(s0, sw)], i0, t0 + c0 + s0, sw)
                    if mode_a:
                        n_copy[0] += 1
                        cwp = 32 if PROBE == "copies" else cw
                        if n_copy[0] % A_COPY_ACT_MOD == 0:
                            nc.scalar.copy(out=em_sb[:, ds(c0, cwp)], in_=ps[:, 0:cwp])
                        else:
                            nc.vector.tensor_copy(em_sb[:, ds(c0, cwp)], ps[:, 0:cwp])
                    else:
                        nc.scalar.activation(
                            out=em_sb[:, ds(c0, cw)], in_=ps[:, 0:cw], func=AF.Exp,
                            bias=negc_sb[:],
                        )
                    yield
                if mode_a:
                    slot, em_dst = a_slot[0], em16_d
                    a_slot[0] += 1
                    shr_sb = wshear_a.tile([128, N], a_sh_t, tag="shr16")
                else:
                    slot, em_dst = b_slot[0], em8_d
                    b_slot[0] += 1
                    shr_sb = wshear_b.tile([128, N], b_shr_t, tag="shr")
                # spread DMA issue across sequencer queues: a shear read
                # head-of-line-waits its em write's completion sem, so on one
                # queue every (write, read) pair serializes with 2x ~900ns
                # sem-prop latency in between
                qmap = {"sp": nc.sync, "act": nc.scalar, "pool": nc.gpsimd}
                wnp = 128 if PROBE == "dma" else WN - 1
                qmap[EMW_Q].dma_start(out=em_dst[ds(slot * 128, 128), 0:wnp],
                                      in_=em_sb[:, 0:wnp])
                shear_ap = bass.AP(em_dst, slot * 128 * WN + 127,
                                   [[WN - 1, 128], [1, 128 if PROBE == "dma" else N]])
                shr_q = nc.gpsimd if ((not mode_a) and B_CAST_READ) else qmap[SHEAR_Q]
                shr_q.dma_start(out=shr_sb[:, 0:(128 if PROBE == "dma" else N)],
                                in_=shear_ap)
                shr_pair.append(shr_sb)
                yield

        def attn_phase(g, shr_pair):
            i0g = g * 256
            mode_a = g in a_pairs
            # pT[dj, J*256 + q*128 + di] = p^T for the pair
            pT_sb = work.tile([128, NJ * 256], BF16, tag="pT")
            step = 4 if CT_BIG else 2   # key tiles per content psum chunk
            cw_ct = step * 256
            if mode_a:
                # content + shifted rel accumulate in PSUM; single Exp pass
                for Jx in range(NJ // step):
                    ps = ppool_ct.tile([128, cw_ct], F32, tag="ps_ct")
                    for u in range(step):
                        J = Jx * step + u
                        for q in range(2):
                            cwq = 8 if PROBE in ("shift", "ct") else 128
                            sl = ps[:, ds(u * 256 + q * 128, cwq)]
                            content_matmul(sl, J, i0g + q * 128, cwq,
                                           start=True, stop=False)
                            nc.tensor.matmul(
                                sl, shr_pair[q][:, ts(J, 128)], ident8[:, 0:cwq],
                                start=False, stop=True,
                            )
                    cwp = 32 if PROBE == "exp" else cw_ct
                    nc.scalar.activation(
                        out=pT_sb[:, ds(Jx * cw_ct, cwp)], in_=ps[:, 0:cwp],
                        func=AF.Exp,
                    )
                    yield
            else:
                # split exp: ecT = exp(content^T); pT = ecT * shr^T
                for Jg2 in range(NJ // 4):
                    ecT_sb = ect_pool.tile([128, 1024], BF16, tag="ecT")
                    for hh in range(1024 // cw_ct):
                        ps = ppool_ct.tile([128, cw_ct], F32, tag="ps_ct")
                        for u in range(step):
                            J = Jg2 * 4 + hh * step + u
                            content_matmul(ps[:, ts(u, 256)], J, i0g, 256,
                                           start=True, stop=True)
                        nc.scalar.activation(
                            out=ecT_sb[:, ds(hh * cw_ct, cw_ct)], in_=ps[:],
                            func=AF.Exp,
                        )
                    ps_t = ppool_st.tile([128, 1024], b_shr_t, tag="ps_st")
                    for u in range(4):
                        J = Jg2 * 4 + u
                        for q in range(2):
                            nc.tensor.transpose(
                                ps_t[:, ds(u * 256 + q * 128, 128)],
                                shr_pair[q][:, ts(J, 128)], ident[:],
                            )
                    nc.vector.tensor_mul(
                        pT_sb[:, ds(Jg2 * 1024, 1024)], ecT_sb[:], ps_t[:]
                    )
                    yield

            # PV + epilogue per q-tile
            for q in range(2):
                i0 = i0g + q * 128
                ps_o = ppool_epi.tile([128, D + 1], F32, tag="ps_epi")
                pvw = 2 if PROBE == "pv" else D + 1
                for J in range(NJ):
                    nc.tensor.matmul(
                        ps_o[:, 0:pvw], pT_sb[:, ds(J * 256 + q * 128, 128)],
                        vext[:, ds(J * (D + 1), pvw)],
                        start=(J == 0), stop=(J == NJ - 1),
                    )
                o_sb = sm.tile([128, D], BF16, tag="o")
                if HOST_NORM:
                    # ship raw row sums; host divides after the W_out matmul
                    I = i0 // 128
                    nc.vector.tensor_copy(sums_sb[:, I:I + 1], ps_o[:, D:D + 1])
                    nc.vector.tensor_copy(o_sb[:], ps_o[:, 0:D])
                else:
                    rc_sb = sm.tile([128, 1], F32, tag="rc")
                    nc.vector.reciprocal(out=rc_sb[:], in_=ps_o[:, D:D + 1])
                    nc.vector.tensor_scalar_mul(o_sb[:], ps_o[:, 0:D], rc_sb[:])
                ps_ot = ppool_epi.tile([D, 128], BF16, tag="ps_epi")
                nc.tensor.transpose(ps_ot[:], o_sb[:], ident[:])
                otT_sb = sm.tile([D, 128], BF16, tag="otT")
                nc.vector.tensor_copy(otT_sb[:], ps_ot[:])
                out_sb = work.tile([128, DIM], BF16, tag="out")
                for w in range(DIM // 512):
                    ps_op = ppool_epi.tile([128, 512], F32, tag="ps_epi")
                    nc.tensor.matmul(
                        ps_op[:], otT_sb[:], wo_sb[:, ts(w, 512)],
                        start=True, stop=True,
                    )
                    n_copy[0] += 1
                    if n_copy[0] % OUT_COPY_ACT_MOD == 0:
                        nc.scalar.copy(out=out_sb[:, ts(w, 512)], in_=ps_op[:])
                    else:
                        nc.vector.tensor_copy(out_sb[:, ts(w, 512)], ps_op[:])
                out_q = {"sp": nc.sync, "act": nc.scalar, "pool": nc.gpsimd}[OUT_Q]
                out_q.dma_start(out=out_d[ds(i0, 128), :], in_=out_sb[:])
                yield

        # Fine-grained software pipeline: interleave the em stream of pair
        # g+1 with the attn stream of pair g, so the copier-paced em segment
        # and the ACT-paced content segment overlap on every engine queue
        # (in-order queues serialize whole phases otherwise).
        P2 = Q // 2
        shr_store = {}
        em_gen = [None]
        attn_gen = [None]
        g_em = [0]
        g_attn = [0]

        def pump_em():
            if em_gen[0] is None:
                if g_em[0] < P2 and g_em[0] <= g_attn[0] + PIPE_DEPTH:
                    lst = []
                    shr_store[g_em[0]] = lst
                    em_gen[0] = em_phase(g_em[0], lst)
                else:
                    return False
            try:
                next(em_gen[0])
            except StopIteration:
                em_gen[0] = None
                g_em[0] += 1
            return True

        def pump_attn():
            if attn_gen[0] is None:
                if g_attn[0] < P2 and g_attn[0] < g_em[0]:
                    attn_gen[0] = attn_phase(g_attn[0], shr_store.pop(g_attn[0]))
                else:
                    return False
            try:
                next(attn_gen[0])
            except StopIteration:
                attn_gen[0] = None
                g_attn[0] += 1
            return True

        while g_attn[0] < P2:
            moved = pump_em()
            for _ in range(INTERLEAVE):
                moved |= pump_attn()
            if not moved:
                raise RuntimeError("pipeline deadlock")

        if HOST_NORM:
            nc.sync.dma_start(out=sums_d.ap(), in_=sums_sb[:])

    if split_waits:
        _patch_tile_drain()
        split_multi_waits(nc)
    return nc


# ---------------- host side ----------------

def get_positional_embed_np(seq_len, feature_size):
    distances = np.arange(-seq_len + 1, seq_len)
    nb = feature_size // 2
    pow_rate = math.exp(math.log(seq_len + 1) / nb)
    center_widths = np.power(np.float32(pow_rate), np.arange(1, nb + 1, dtype=np.float32)) - 1.0
    emb = (center_widths[None, :] > np.abs(distances)[:, None]).astype(np.float32)
    signed = np.sign(distances).astype(np.float32)[:, None] * emb
    return np.concatenate([emb, signed], axis=-1)  # [2n-1, F]


def make_in_maps(x, W_q, W_k, W_v, W_rel_k, W_out, rel_content_bias, rel_pos_bias):
    B, N, _ = np.asarray(x).shape
    PW = 2 * N
    f16 = np.float16
    import ml_dtypes
    bf16 = ml_dtypes.bfloat16
    xT = np.ascontiguousarray(np.asarray(x[0], np.float32).T).astype(f16)
    pos = get_positional_embed_np(N, np.asarray(W_rel_k).shape[0])
    posT = np.zeros((192, PW), np.float32)
    posT[:, : 2 * N - 1] = pos.T
    posT = posT.astype(f16)
    x0 = np.asarray(x[0], np.float32)
    in_maps = []
    for h in range(H):
        sl = slice(h * D, (h + 1) * D)
        bc_h = np.asarray(rel_content_bias, np.float32)[0, h, 0, :]
        bck = x0 @ (np.asarray(W_k, np.float32)[:, sl] @ bc_h)  # [N]
        ebck = np.exp(bck - bck.max()).astype(np.float32)
        in_maps.append({
            "xT": xT,
            "posT": posT,
            "ebck": np.ascontiguousarray(ebck.reshape(-1, 128).T),
            "wq": np.ascontiguousarray(np.asarray(W_q)[:, sl]).astype(f16),
            "wk": np.ascontiguousarray(np.asarray(W_k)[:, sl]).astype(f16),
            "wv": np.ascontiguousarray(np.asarray(W_v)[:, sl]).astype(f16),
            "wrk": np.ascontiguousarray(np.asarray(W_rel_k)[:, sl]).astype(f16),
            "wo": np.ascontiguousarray(np.asarray(W_out)[sl, :]).astype(bf16),
            "bc": np.ascontiguousarray(
                np.asarray(rel_content_bias, np.float32)[0, h, 0, :].reshape(D, 1)),
            "bp": np.ascontiguousarray(
                np.asarray(rel_pos_bias, np.float32)[0, h, 0, :].reshape(D, 1)),
        })
    return in_maps


def combine_outputs(results, b_out):
    acc = None
    for r in results:
        p = r["out"].astype(np.float32)
        if HOST_NORM:
            s = np.asarray(r["sums"], np.float32).T.reshape(-1)  # [N]
            p = p / s[:, None]
        acc = p if acc is None else acc + p
    acc = acc + np.asarray(b_out, np.float32)[None, :]
    return acc[None]  # [1, N, DIM]


# ---------------- entry point ----------------

_NC_CACHE = {}


def kernel(x, W_q, W_k, W_v, W_rel_k, W_out, b_out,
           rel_content_bias, rel_pos_bias):
    """Full-input entry: shards per head across 8 NeuronCores, returns the
    full [1, N, 1536] float32 output."""
    from concourse import bass_utils

    x = np.asarray(x)
    N = x.shape[1]
    if N not in _NC_CACHE:
        _NC_CACHE[N] = build(N)
    nc = _NC_CACHE[N]
    in_maps = make_in_maps(x, W_q, W_k, W_v, W_rel_k, W_out,
                           rel_content_bias, rel_pos_bias)
    res = bass_utils.run_bass_kernel_spmd(nc, in_maps, core_ids=list(range(H)))
    return combine_outputs(res.results, b_out).astype(np.float32)
